# revision 10
# baseline (speedup 1.0000x reference)
"""GCN layer (dense projection + sparse neighbor aggregation) on 8 Trainium2
NeuronCores via Bass/Tile.

Strategy: shard nodes (and their incident edges, grouped by destination row)
across the 8 cores; replicate W/b; AllGather the projected node features in
fp16 (halves the collective window) into 3 int16-addressable sub-tables of
256-byte row-pairs; per 128-row output block, bulk-gather the needed source
pairs with DMAGatherAnt (edges parity-sorted so each chunk's matmul slices
the correct half), scale by edge_val on DVE, and segment-sum via an
assignment-matrix matmul accumulated in PSUM (bias folded in as an extra
rank-128 matmul; padded gather slots are killed by rowloc=-1).

The gather stream and the collectives share the 16 SDMA engines and starve
each other, so the schedule serializes them: a tiny warm-up collective
absorbs the ~60us NRT launch latency, bucket 0 (one group) gathers overlap
only the tail of the collective window, and a dummy gpsimd read of the last
sub-table fences the remaining gathers behind the final AllGather.
"""

import sys

if "/opt/trn_rl_repo" not in sys.path:
    sys.path.insert(0, "/opt/trn_rl_repo")

import numpy as np

import concourse.bass as bass
import concourse.mybir as mybir
import concourse.tile as tile
from concourse import bacc
from concourse.bass_utils import run_bass_kernel_spmd

N_NODES = 100000
N_EDGES = 1600000
IN_FT = 256
OUT_FT = 64
NCORES = 8
NS = N_NODES // NCORES          # 12500 nodes per core
NB = (NS + 127) // 128          # 98 row blocks per core
NSP = NB * 128                  # 12544 padded nodes per core
GB = 7                          # row blocks per group (98 = 14 * 7)
NGROUPS = NB // GB              # 14
# buckets = sub-tables, one fp16 AllGather each; bucket 0 is a single group
# so its gathers can overlap the remaining collectives
BGROUPS = [[0], [1, 2, 3, 4, 5, 6], [7, 8, 9, 10, 11, 12, 13]]
NBUCKET = len(BGROUPS)
BUCKET_OF_G = [b for b, gs in enumerate(BGROUPS) for _ in gs]
BBLK0 = [gs[0] * GB for gs in BGROUPS]          # first block per bucket
BNBLK = [len(gs) * GB for gs in BGROUPS]        # blocks per bucket
BROWS = [NCORES * 128 * n for n in BNBLK]       # rows per sub-table
BPAIRS = [r // 2 for r in BROWS]                # 256B fp16 row-pairs

F32 = mybir.dt.float32
F16 = mybir.dt.float16
I32 = mybir.dt.int32
I16 = mybir.dt.int16

MAXCH = 8                       # 1024 indices = HW cap per dma_gather
NQ = 4                          # SWDGE queues (set to 1 for CoreSim runs)


def _chunk_layout(cap):
    """Stream order: bucket-major, then group, then block, then parity.

    cap[blk][bkt][par] chunks per cell.  Returns (start[blk][bkt][par],
    tot_chunks, region[(bkt, g)] = (lo, hi))."""
    cap = np.asarray(cap)
    start = np.zeros((NB, NBUCKET, 2), np.int64)
    region = {}
    off = 0
    for bkt in range(NBUCKET):
        for g in range(NGROUPS):
            lo = off
            for j in range(GB):
                blk = g * GB + j
                for par in range(2):
                    start[blk, bkt, par] = off
                    off += int(cap[blk, bkt, par])
            region[(bkt, g)] = (lo, off)
    return start, int(off), region


def build_program(cap):
    """One SPMD Bass program; all 8 cores run it on their own shards."""
    cap = np.asarray(cap)                        # [NB, NBUCKET, 2]
    start, tot, region = _chunk_layout(cap)
    gcols = tot * 8

    nc = bacc.Bacc("TRN2", target_bir_lowering=False, debug=False,
                   num_devices=NCORES, num_swdge_queues=NQ)

    seqT = nc.dram_tensor("seqT", [2, 128, NSP], F32, kind="ExternalInput")
    gidx = nc.dram_tensor("gidx", [128, gcols], I16, kind="ExternalInput")
    val = nc.dram_tensor("val", [128, tot], F16, kind="ExternalInput")
    rl = nc.dram_tensor("rl", [128, tot], F16, kind="ExternalInput")
    w_in = nc.dram_tensor("w", [128, 2, OUT_FT], F32, kind="ExternalInput")
    bias_in = nc.dram_tensor("biasb", [128, OUT_FT], F16,
                             kind="ExternalInput")
    # partition-major layouts: [p, block, feature]; host un-permutes
    sf_out = nc.dram_tensor("sf", [128, NB, OUT_FT], F32,
                            kind="ExternalOutput")
    agg_out = nc.dram_tensor("agg", [128, NB, OUT_FT], F32,
                             kind="ExternalOutput")
    ccin = [nc.dram_tensor(f"ccin{b}", [128, BNBLK[b], OUT_FT], F16)
            for b in range(NBUCKET)]
    xt = [nc.dram_tensor(f"xt{b}", [BPAIRS[b], 2 * OUT_FT], F16,
                         addr_space="Shared") for b in range(NBUCKET)]
    warm_in = nc.dram_tensor("warm_in", [16, OUT_FT], F32)
    warm_out = nc.dram_tensor("warm_out", [16 * NCORES, OUT_FT], F32,
                              addr_space="Shared")

    groups = [list(range(NCORES))]
    qrr = [0]

    with tile.TileContext(nc) as tc:
        with (
            tc.tile_pool(name="const", bufs=1) as cpool,
            tc.tile_pool(name="psum", bufs=2, space="PSUM") as psum_pool,
            tc.tile_pool(name="seqpool", bufs=4) as seqpool,
            tc.tile_pool(name="p1work", bufs=3) as p1work,
            tc.tile_pool(name="accpool", bufs=1) as accpool,
            tc.tile_pool(name="edgemeta", bufs=1) as mpool,
            tc.tile_pool(name="p2work", bufs=2) as p2,
        ):
            w_sb = cpool.tile([128, 2, OUT_FT], F32)
            nc.sync.dma_start(out=w_sb[:], in_=w_in[:])
            bias_sb = cpool.tile([128, OUT_FT], F16)
            nc.sync.dma_start(out=bias_sb[:], in_=bias_in[:])
            ones_sb = cpool.tile([128, 128], F16)
            nc.gpsimd.memset(ones_sb[:], 1.0)
            iota_i = cpool.tile([128, 128], I32)
            nc.gpsimd.iota(iota_i[:], pattern=[[1, 128]], base=0,
                           channel_multiplier=0)
            iota_f = cpool.tile([128, 128], F16)
            nc.vector.tensor_copy(out=iota_f[:], in_=iota_i[:])

            # tiny collective up front absorbs the NRT first-collective
            # barrier + launch latency off the critical path
            nc.gpsimd.collective_compute(
                "AllGather",
                mybir.AluOpType.bypass,
                replica_groups=groups,
                ins=[warm_in[:]],
                outs=[warm_out[:]],
            )

            # edge metadata on the scalar HWDGE ring: ready before the first
            # gather, independent of the seqT loads on the sync ring
            acc = accpool.tile([128, NB, OUT_FT], F32)
            gidx_sb = mpool.tile([128, gcols], I16)
            nc.scalar.dma_start(out=gidx_sb[:], in_=gidx[:])
            val_sb = mpool.tile([128, tot], F16)
            nc.scalar.dma_start(out=val_sb[:], in_=val[:])
            rl_sb = mpool.tile([128, tot], F16)
            nc.scalar.dma_start(out=rl_sb[:], in_=rl[:])

            # ---- phase 1: x = seq @ W (fp32); fp16 ccin staged ----
            for g in range(NGROUPS):
                sq = seqpool.tile([128, 2, GB * 128], F32, tag="sq")
                for kc in range(2):
                    nc.sync.dma_start(
                        out=sq[:, kc, :],
                        in_=seqT[kc, :, g * GB * 128:(g + 1) * GB * 128])
                x_sb = p1work.tile([128, GB, OUT_FT], F32, tag="x_sb")
                c16 = p1work.tile([128, GB, OUT_FT], F16, tag="c16")
                for j in range(GB):
                    px = psum_pool.tile([128, OUT_FT], F32, tag="px")
                    for kc in range(2):
                        nc.tensor.matmul(
                            px[:],
                            sq[:, kc, j * 128:(j + 1) * 128],
                            w_sb[:, kc, :],
                            start=(kc == 0),
                            stop=(kc == 1),
                        )
                    nc.vector.tensor_copy(out=x_sb[:, j, :], in_=px[:])
                    nc.scalar.activation(
                        out=c16[:, j, :], in_=px[:],
                        func=mybir.ActivationFunctionType.Copy)
                nc.scalar.dma_start(
                    out=sf_out[:, g * GB:(g + 1) * GB, :], in_=x_sb[:])
                b_ = BUCKET_OF_G[g]
                g0 = (g - BGROUPS[b_][0]) * GB
                nc.scalar.dma_start(
                    out=ccin[b_][:, g0:g0 + GB, :], in_=c16[:])

            # ---- phase 2 ----
            # all AllGather triggers first (the CC stream runs them serially
            # as their inputs land), then bucket-0 gathers, then a fence on
            # the last sub-table before the remaining gathers
            for bkt in range(NBUCKET):
                nc.gpsimd.collective_compute(
                    "AllGather",
                    mybir.AluOpType.bypass,
                    replica_groups=groups,
                    ins=[ccin[bkt][:]],
                    outs=[xt[bkt][:]],
                )

            fence_sb = cpool.tile([128, 16], F16)

            def do_bucket(bkt):
                for g in range(NGROUPS):
                    lo, hi = region[(bkt, g)]
                    nreg = hi - lo
                    xg = p2.tile([128, nreg, 2 * OUT_FT], F16, tag="xg")
                    c0 = lo
                    while c0 < hi:
                        ln = min(MAXCH, hi - c0)
                        nc.gpsimd.dma_gather(
                            out_ap=xg[:, c0 - lo:c0 - lo + ln, :],
                            in_ap=xt[bkt][:],
                            idxs_ap=gidx_sb[:, c0 * 8:(c0 + ln) * 8],
                            num_idxs=ln * 128,
                            num_idxs_reg=ln * 128,
                            elem_size=2 * OUT_FT,
                            queue_num=qrr[0] % NQ,
                        )
                        qrr[0] += 1
                        c0 += ln
                    # scale both pair-halves by edge_val in one DVE pass
                    msg = p2.tile([128, nreg, 2 * OUT_FT], F16, tag="msg")
                    nc.vector.tensor_tensor(
                        out=msg[:],
                        in0=xg[:],
                        in1=val_sb[:, lo:hi].unsqueeze(2).broadcast_to(
                            [128, nreg, 2 * OUT_FT]),
                        op=mybir.AluOpType.mult,
                    )
                    o_sb = None
                    if bkt == NBUCKET - 1:
                        o_sb = p2.tile([128, GB, OUT_FT], F32, tag="o_sb")
                    for j in range(GB):
                        blk = g * GB + j
                        nch = int(cap[blk, bkt, 0] + cap[blk, bkt, 1])
                        s = int(start[blk, bkt, 0])
                        a_sb = p2.tile([128, nch * 128], F16, tag="a_sb")
                        nc.vector.tensor_tensor(
                            out=a_sb[:].rearrange("p (c q) -> p c q", q=128),
                            in0=rl_sb[:, s:s + nch].unsqueeze(
                                2).broadcast_to([128, nch, 128]),
                            in1=iota_f[:].unsqueeze(1).broadcast_to(
                                [128, nch, 128]),
                            op=mybir.AluOpType.is_equal,
                        )
                        po = psum_pool.tile([128, OUT_FT], F32, tag="po")
                        if bkt == 0:
                            nc.tensor.matmul(po[:], ones_sb[:], bias_sb[:],
                                             start=True, stop=False)
                        for cc in range(nch):
                            ci = s + cc
                            par = int(cc >= cap[blk, bkt, 0])
                            nc.tensor.matmul(
                                po[:],
                                a_sb[:, cc * 128:(cc + 1) * 128],
                                msg[:, ci - lo,
                                    par * OUT_FT:(par + 1) * OUT_FT],
                                start=(bkt != 0 and cc == 0),
                                stop=(cc == nch - 1),
                            )
                        if bkt == 0:
                            nc.vector.tensor_copy(out=acc[:, blk, :],
                                                  in_=po[:])
                        elif bkt < NBUCKET - 1:
                            nc.vector.tensor_tensor(
                                out=acc[:, blk, :], in0=acc[:, blk, :],
                                in1=po[:], op=mybir.AluOpType.add)
                        else:
                            nc.vector.tensor_tensor(
                                out=o_sb[:, j, :], in0=acc[:, blk, :],
                                in1=po[:], op=mybir.AluOpType.add)
                            nc.scalar.activation(
                                out=o_sb[:, j, :], in_=o_sb[:, j, :],
                                func=mybir.ActivationFunctionType.Relu)
                    if bkt == NBUCKET - 1:
                        nc.sync.dma_start(
                            out=agg_out[:, g * GB:(g + 1) * GB, :],
                            in_=o_sb[:])

            do_bucket(0)
            # fence: the gpsimd in-order queue holds later gathers here until
            # the last AllGather has landed, so gathers and collectives never
            # contend for the SDMA engines
            nc.gpsimd.dma_start(out=fence_sb[:, 0:16],
                                in_=xt[NBUCKET - 1][0:128, 0:16])
            for bkt in range(1, NBUCKET):
                do_bucket(bkt)

    nc.compile()
    return nc


def prepare_inputs(seq, edge_row, edge_col, edge_val, W, b):
    """Host-side sharding / graph partitioning. Returns (in_maps, caps)."""
    seq = np.asarray(seq, dtype=np.float32).reshape(N_NODES, IN_FT)
    r = np.asarray(edge_row).astype(np.int64)
    c = np.asarray(edge_col).astype(np.int64)
    v = np.asarray(edge_val, dtype=np.float32)
    W = np.asarray(W, dtype=np.float32).reshape(IN_FT, OUT_FT)
    b = np.asarray(b, dtype=np.float32).reshape(OUT_FT)

    # destination side
    core = r // NS
    loc = r - core * NS
    blk = loc >> 7
    rowloc = (loc & 127).astype(np.float16)
    # source side: fp16 row-pair index within its sub-table + pair half
    csrc = c // NS
    crem = c % NS
    cblk = crem // 128
    cp = crem % 128
    g_of_blk = np.array([BUCKET_OF_G[blk_ // GB] for blk_ in range(NB)])
    bucket = g_of_blk[cblk]
    bnblk = np.array(BNBLK)[bucket]
    bblk0 = np.array(BBLK0)[bucket]
    rowr = csrc * (128 * bnblk) + cp * bnblk + (cblk - bblk0)
    lidx = (rowr // 2).astype(np.int16)
    par = (rowr % 2).astype(np.int64)

    # per-(core, block, bucket, parity) counts -> caps (max over cores)
    key = ((core * NB + blk) * NBUCKET + bucket) * 2 + par
    ngrp = NCORES * NB * NBUCKET * 2
    counts = np.bincount(key, minlength=ngrp).reshape(
        NCORES, NB, NBUCKET, 2)
    cap = np.maximum(1, -(-counts.max(axis=0) // 128))     # [NB, NBUCKET, 2]

    startc, tot, _ = _chunk_layout(cap)

    order = np.argsort(key, kind="stable")
    key_s = key[order]
    starts = np.searchsorted(key_s, np.arange(ngrp))
    pos = np.arange(N_EDGES) - starts[key_s]
    kp = key_s % 2
    kb = (key_s // 2) % NBUCKET
    kblk = (key_s // (2 * NBUCKET)) % NB
    kcore = key_s // (2 * NBUCKET * NB)
    dest = kcore * (tot * 128) + startc[kblk, kb, kp] * 128 + pos

    idxp = np.zeros(NCORES * tot * 128, np.int16)        # pad: pair 0
    valp = np.zeros(NCORES * tot * 128, np.float16)
    rlp = np.full(NCORES * tot * 128, -1.0, np.float16)  # pad: killed
    idxp[dest] = lidx[order]
    valp[dest] = v[order].astype(np.float16)
    rlp[dest] = rowloc[order]

    idxp = idxp.reshape(NCORES, tot, 128)
    valp = valp.reshape(NCORES, tot, 128)
    rlp = rlp.reshape(NCORES, tot, 128)

    val_arr = np.ascontiguousarray(valp.transpose(0, 2, 1))
    rl_arr = np.ascontiguousarray(rlp.transpose(0, 2, 1))
    wi = idxp.reshape(NCORES, tot * 8, 16).transpose(0, 2, 1)
    gidx_full = np.broadcast_to(wi[:, None], (NCORES, 8, 16, tot * 8))
    gidx_full = np.ascontiguousarray(
        gidx_full.reshape(NCORES, 128, tot * 8))

    biasb = np.broadcast_to((b / 128.0).astype(np.float16),
                            (128, OUT_FT)).copy()
    w3 = np.ascontiguousarray(
        W.reshape(2, 128, OUT_FT).transpose(1, 0, 2))  # [128, 2, OUT_FT]

    in_maps = []
    for k in range(NCORES):
        shard = np.zeros((NSP, IN_FT), np.float32)
        shard[:NS] = seq[k * NS:(k + 1) * NS]
        seqT_k = np.ascontiguousarray(shard.T).reshape(2, 128, NSP)
        in_maps.append({
            "seqT": seqT_k,
            "gidx": gidx_full[k],
            "val": val_arr[k],
            "rl": rl_arr[k],
            "w": w3,
            "biasb": biasb,
        })
    return in_maps, tuple(map(tuple, cap.reshape(NB, NBUCKET * 2).tolist()))


_PROGRAMS: dict[tuple, object] = {}


def kernel(seq, edge_row, edge_col, edge_val, W, b):
    in_maps, caps = prepare_inputs(seq, edge_row, edge_col, edge_val, W, b)
    prog = _PROGRAMS.get(caps)
    if prog is None:
        cap = np.asarray(caps).reshape(NB, NBUCKET, 2)
        prog = _PROGRAMS[caps] = build_program(cap)
    res = run_bass_kernel_spmd(prog, in_maps, core_ids=list(range(NCORES)))

    def unshard(name):
        # [128, NB, OUT_FT] partition-major -> [NS, OUT_FT] row-major
        parts = [
            res.results[k][name].transpose(1, 0, 2).reshape(NSP, OUT_FT)[:NS]
            for k in range(NCORES)
        ]
        return np.concatenate(parts)[None]

    return unshard("agg"), unshard("sf")
